# revision 18
# baseline (speedup 1.0000x reference)
"""Channel-attention transformer block on 8 Trainium2 NeuronCores.

Reference semantics (b=8, l=4096, c=512, h=8 heads carved from the
*sequence* axis, head_pos = l % 8):
    qkv = x @ w_qkv.T ; split q,k,v per head  (each (lh=512, c=512))
    attn = softmax((q.T @ k) / 8, axis=-1)    # (c, c) channel attention
    y.T  = attn @ v.T                         # (c, lh)
    out  = y @ w_out.T + b_out
Sharding: data-parallel over batch — core i handles batch i.

Per-core layout trick: the l axis is permuted on the host so each head's
512 rows are contiguous (row h*512+i <- original row i*8+h), and x is
shipped per-head transposed with per-partition-contiguous blocks
xh[h][p][ko][l] (c = ko*128 + p), so every x DMA is 128 descriptors of
contiguous 8 KiB (vs 512x512B for a plain (c,l) layout). Then per head:
  - Q,K in natural (l, c) layout and V^T in (c, l) layout all come
    straight out of matmuls against xh (no on-device transposes),
  - scores are computed *transposed* (S^T = K^T Q via lhsT=K, rhs=Q) so
    softmax's sum over the attended axis lands on the partition dim,
    where it is computed by a matmul against ones columns glued onto
    V^T (columns 0-1 of the AV rhs) — again no transposes,
  - normalization (multiply by 1/denominator, a per-partition scalar)
    is fused into the PSUM->SBUF evacuation of the AV result,
  - the out-projection consumes y^T (c on partitions) directly as lhsT.
The host un-permutes rows of the returned (4096, 512) per-core output.

Startup choreography (the PE stream is ~99% dense once running, so the
wins are at the edges): DIRECT2D descriptor generation serializes per
issuing queue at ~0.6-1.0 us per dma_start, so the first-needed pieces
(head-0 x split per ko, q/k weight pieces) are interleaved across BOTH
HWDGE queues (sync + scalar) in deadline order, and the QK projection
iterates ko-outer/m-inner (interleaved PSUM accumulation groups) so the
first matmul needs only ONE x piece and ONE weight piece. ~3us of
warmup matmuls on zeros bridge the PE p-state ramp while the first
operands are still in flight. The out-projection is also ko-outer for
heads 0-6 so the matmul consuming the last-normalized y strip has
~2.6us of buffered PE work in front of it (no idle gap -> no p-state
dip). All DMA stays on the two HWDGE queues (SWDGE unused, which drops
its 8 semaphores from the preamble init and the teardown storm);
output stores alternate sync/scalar and the final strip is computed as
two 256-col groups so its first half stores while the second half is
still in the PE.
"""

import numpy as np

import concourse.bass as bass
import concourse.mybir as mybir
import concourse.tile as tile
from concourse.bass_utils import run_bass_kernel_spmd

B = 8
L = 4096
C = 512
HEADS = 8
LH = L // HEADS  # 512
SCALE = 64 ** -0.5  # DIM_HEAD ** -0.5 from the reference
N_CORES = 8
P = 128
KC = C // P  # 4 contraction chunks of 128
F32 = mybir.dt.float32

# Matmul operand dtype: bf16 streams 1 col/cycle through the PE — the same
# throughput as fp32r — but halves every input DMA byte (the startup is
# DMA-ring-bandwidth-bound), halves SBUF traffic, and doubles DVE
# evacuation speed. Accuracy: bf16 rounding is 4x coarser than fp32r's
# TF32, lifting rel err from ~5e-4 to ~2e-3 — far inside the 2e-2 gate.
# PSUM accumulation stays fp32.
MM_DTYPE = mybir.dt.bfloat16
PD = MM_DTYPE  # dtype of every tile that feeds a matmul


def _split_wide_waits(nc, max_waits=1):
    """This container's walrus build rejects instructions carrying more than
    ~1 sync wait ("Too many sync wait commands", e.g. in the S3_LW lowering
    of a fused matmul). Hoist surplus waits onto same-engine nops inserted
    immediately before the offending instruction — the engine stalls at the
    same point in its stream, so scheduling semantics are unchanged."""
    for f in nc.m.functions:
        for bb in f.blocks:
            snapshot = list(bb.instructions)
            if not any(
                inst.sync_info and inst.sync_info.on_wait
                and len(inst.sync_info.on_wait) > max_waits
                for inst in snapshot
            ):
                continue
            new = []
            for inst in snapshot:
                si = inst.sync_info
                waits = list(si.on_wait) if si and si.on_wait else []
                if len(waits) > max_waits:
                    for w in waits[:-max_waits]:
                        nop = nc.engines[inst.engine].nop(nofuse=True).ins
                        cur = nc.cur_bb.bb.instructions
                        assert cur[-1] is nop
                        cur.pop()  # re-homed below, right before `inst`
                        nop.sync_info = mybir.SyncInfo(on_wait=[w], on_update=[])
                        new.append(nop)
                    inst.sync_info = mybir.SyncInfo(
                        on_wait=waits[-max_waits:],
                        on_update=list(si.on_update) if si.on_update else [],
                    )
                new.append(inst)
            bb.instructions = new


def _emit(ctx, tc, xh, wqkv_t, wout_t, out):
    """Emit the per-core program. All DRAM APs:
    xh (HEADS, P, KC, LH) fp32 (per-head transposed x, partition-major),
    wqkv_t (C, 3C) fp32 (q block pre-scaled), wout_t (C, C),
    out (L, C)."""
    nc = tc.nc
    EXP = mybir.ActivationFunctionType.Exp

    xh_r = xh.rearrange("h p ko l -> p h ko l")
    wqkv_r = wqkv_t.rearrange("(ko p) n -> p ko n", p=P)
    wout_r = wout_t.rearrange("(ko p) n -> p ko n", p=P)

    consts = ctx.enter_context(tc.tile_pool(name="consts", bufs=1))
    # bufs=2 (not 3) doubles as a DMA-ring throttle: head h+1's x load
    # acquires the buffer of head h-1, so it can't hit the rings until
    # V-proj(h-1) is done — keeping the startup-critical weight/x pieces
    # from being starved by background prefetch traffic.
    xt_pool = ctx.enter_context(tc.tile_pool(name="xt", bufs=2))
    q_pool = ctx.enter_context(tc.tile_pool(name="q", bufs=3))
    k_pool = ctx.enter_context(tc.tile_pool(name="k", bufs=3))
    vt_pool = ctx.enter_context(tc.tile_pool(name="vt", bufs=3))
    exp_pool = ctx.enter_context(tc.tile_pool(name="exp", bufs=3))
    y_pool = ctx.enter_context(tc.tile_pool(name="y", bufs=3))
    out_pool = ctx.enter_context(tc.tile_pool(name="out", bufs=8))
    recip_pool = ctx.enter_context(tc.tile_pool(name="recip", bufs=8))
    pp_mm = ctx.enter_context(tc.tile_pool(name="pp_mm", bufs=8, space="PSUM"))

    wqkv = consts.tile([P, KC, 3 * C], PD)
    wout = consts.tile([P, KC, C], PD)
    xth0 = xt_pool.tile([P, KC, LH], PD, tag="xth")

    # PE warmup: ~3.2us of small matmuls on zeros, overlapping the DMA
    # lead-in, so the PE p-state ramp (full speed only after ~3us of
    # continuous execution) completes right as the first real operands
    # land (~11.2us). 256-col units keep the warmup->real handoff fine-
    # grained so real work is not queued behind a long warmup op.
    wu = consts.tile([P, 2 * P], PD)
    nc.vector.memset(wu[:], 0.0)
    pwu = pp_mm.tile([P, 2 * P], F32, tag="mm")
    for _ in range(22):
        nc.tensor.matmul(pwu[:], wu[:, 0:P], wu[:], start=True, stop=True)

    def ld_x0(eng, ko):
        eng.dma_start(xth0[:, ko, :], xh_r[:, 0, ko, :])

    def ld_w(eng, j, ko):
        eng.dma_start(wqkv[:, ko, bass.ts(j, C)], wqkv_r[:, ko, bass.ts(j, C)])

    # Deadline-ordered startup pieces, interleaved across the two HWDGE
    # queues. With ko-outer QK groups the PE consumes (x0[ko], wq[ko])
    # pairs every ~0.9 us, matching each queue's ~0.6 us/piece
    # descriptor-generation cadence.
    ld_x0(nc.sync, 0)
    ld_w(nc.sync, 0, 0)
    ld_w(nc.scalar, 0, 1)
    ld_x0(nc.scalar, 1)
    ld_x0(nc.sync, 2)
    ld_w(nc.sync, 0, 2)
    ld_w(nc.scalar, 0, 3)
    ld_x0(nc.scalar, 3)
    ld_w(nc.sync, 1, 0)
    ld_w(nc.scalar, 1, 1)
    ld_w(nc.sync, 1, 2)
    ld_w(nc.scalar, 1, 3)
    ld_w(nc.sync, 2, 0)   # v-block weights, needed from ~T0+7.3us
    ld_w(nc.scalar, 2, 1)
    ld_w(nc.sync, 2, 2)
    ld_w(nc.scalar, 2, 3)
    nc.scalar.dma_start(wout[:], wout_r[:])  # needed from ~T0+22us
    # Head 1's x on the sync queue AFTER every startup-critical piece:
    # queue position defers its ring traffic past the startup crunch
    # (it isn't needed until ~T0+21us).
    xth1 = xt_pool.tile([P, KC, LH], PD, tag="xth")
    nc.sync.dma_start(xth1[:], xh_r[:, 1, :, :])

    for h in range(HEADS):
        if h == 0:
            xth = xth0
        elif h == 1:
            xth = xth1
        else:
            # Alternate the HWDGE queues for the x prefetch: keeps SWDGE
            # (gpsimd) completely unused, which drops the 8 DMASW
            # semaphores from the preamble init and teardown storm. The
            # WAR wait on the xt buffer (V-proj of head h-2) has long
            # released by the time the queue reaches this instruction.
            xth = xt_pool.tile([P, KC, LH], PD, tag="xth")
            eng = nc.sync if h % 2 == 0 else nc.scalar
            eng.dma_start(xth[:], xh_r[:, h, :, :])

        # ---- projections: Q,K natural (l, c); V^T (c, l) with ones col ----
        # ko-outer with 4 interleaved PSUM accumulation groups (one per l'
        # strip m): the first matmul of head 0 depends on just one x piece
        # and one weight piece instead of all four.
        q = q_pool.tile([P, KC, C], PD)
        k = k_pool.tile([P, KC, C], PD)
        for j, dst in ((0, q), (1, k)):
            pqs = [pp_mm.tile([P, C], F32, tag="mm", name=f"pq{j}_{m}")
                   for m in range(KC)]
            for ko in range(KC):
                for m in range(KC):
                    nc.tensor.matmul(
                        pqs[m][:], xth[:, ko, bass.ts(m, P)],
                        wqkv[:, ko, bass.ts(j, C)],
                        start=(ko == 0), stop=(ko == KC - 1))
            for m in range(KC):
                nc.vector.tensor_copy(dst[:, m, :], pqs[m][:])

        vt = vt_pool.tile([P, KC, LH + 2], PD)
        nc.vector.memset(vt[:, :, 0:2], 1.0)
        for m in range(KC):  # c_v strips of 128
            pv = pp_mm.tile([P, LH], F32, tag="mm")
            for ko in range(KC):
                nc.tensor.matmul(
                    pv[:], wqkv[:, ko, bass.ds(2 * C + m * P, P)],
                    xth[:, ko, :],
                    start=(ko == 0), stop=(ko == KC - 1))
            # Evacuate on the scalar (Activation) engine: during the V
            # stage the vector queue already carries the k-stage CASTs
            # (~2.8us) and adding vt would oversubscribe it (5.6us of DVE
            # work in a 3.5us window); scalar idles here.
            nc.scalar.activation(vt[:, m, 2:LH + 2], pv[:],
                                 mybir.ActivationFunctionType.Copy)

        # ---- scores transposed + exp:  S^T[d, c] = sum_l K[l,d] Q[l,c] ----
        ex = exp_pool.tile([P, KC, C], PD)
        for ds_ in range(KC):  # d strips of 128
            ps = pp_mm.tile([P, C], F32, tag="mm")
            for m in range(KC):  # contraction over l' chunks
                nc.tensor.matmul(
                    ps[:], k[:, m, bass.ts(ds_, P)],
                    q[:, m, :],
                    start=(m == 0), stop=(m == KC - 1))
            nc.scalar.activation(ex[:, ds_, :], ps[:], EXP)

        # ---- AV with fused denominator (rhs cols 0,1 are ones; the 514
        # output columns are split 258+256 because a matmul dst cannot
        # exceed one PSUM bank = 512 fp32) ----
        NY1 = 258  # 2 (denominator twice) + 256 v columns
        NY2 = 256
        y = y_pool.tile([P, KC, LH], PD)
        for cs in range(KC):  # c strips of 128
            py1 = pp_mm.tile([P, NY1], F32, tag="mm")
            py2 = pp_mm.tile([P, NY2], F32, tag="mm")
            for ko in range(KC):  # contraction over d chunks
                lhsT = ex[:, ko, bass.ts(cs, P)]
                nc.tensor.matmul(py1[:], lhsT, vt[:, ko, 0:NY1],
                                 start=(ko == 0), stop=(ko == KC - 1))
            for ko in range(KC):
                lhsT = ex[:, ko, bass.ts(cs, P)]
                nc.tensor.matmul(py2[:], lhsT, vt[:, ko, NY1:LH + 2],
                                 start=(ko == 0), stop=(ko == KC - 1))
            rc = recip_pool.tile([P, 1], F32)
            nc.vector.reciprocal(rc[:], py1[:, 0:1])
            nc.vector.tensor_scalar_mul(y[:, cs, 0:NY1 - 2], py1[:, 2:NY1], rc[:])
            nc.vector.tensor_scalar_mul(y[:, cs, NY1 - 2:LH], py2[:], rc[:])

        # ---- out projection: out[l, co] = sum_c y^T[c, l] woutT[c, co] ----
        if h < HEADS - 1:
            pos = [pp_mm.tile([P, C], F32, tag="mm", name=f"po_{m}")
                   for m in range(KC)]
            for ko in range(KC):
                for m in range(KC):
                    nc.tensor.matmul(
                        pos[m][:], y[:, ko, bass.ts(m, P)],
                        wout[:, ko, :],
                        start=(ko == 0), stop=(ko == KC - 1))
            for m in range(KC):
                ot = out_pool.tile([P, C], PD)
                nc.vector.tensor_copy(ot[:], pos[m][:])
                eng = nc.sync if m % 2 == 0 else nc.scalar
                eng.dma_start(out[bass.ds(h * LH + m * P, P), :], ot[:])
        else:
            for m in range(KC):  # l' strips of 128
                rows = bass.ds(h * LH + m * P, P)
                if m == KC - 1:
                    # Final strip: two 256-col accumulation groups so the
                    # first half evacuates + stores while the second half's
                    # matmuls still run; halves go out on both HWDGE queues.
                    pa = pp_mm.tile([P, 256], F32, tag="mm")
                    pb = pp_mm.tile([P, 256], F32, tag="mm")
                    ot = out_pool.tile([P, C], PD)
                    for ko in range(KC):
                        nc.tensor.matmul(
                            pa[:], y[:, ko, bass.ts(m, P)],
                            wout[:, ko, 0:256],
                            start=(ko == 0), stop=(ko == KC - 1))
                    nc.vector.tensor_copy(ot[:, 0:256], pa[:])
                    nc.sync.dma_start(out[rows, 0:256], ot[:, 0:256])
                    for ko in range(KC):
                        nc.tensor.matmul(
                            pb[:], y[:, ko, bass.ts(m, P)],
                            wout[:, ko, 256:512],
                            start=(ko == 0), stop=(ko == KC - 1))
                    nc.vector.tensor_copy(ot[:, 256:512], pb[:])
                    nc.scalar.dma_start(out[rows, 256:512], ot[:, 256:512])
                else:
                    po = pp_mm.tile([P, C], F32, tag="mm")
                    for ko in range(KC):
                        nc.tensor.matmul(
                            po[:], y[:, ko, bass.ts(m, P)],
                            wout[:, ko, :],
                            start=(ko == 0), stop=(ko == KC - 1))
                    ot = out_pool.tile([P, C], PD)
                    nc.vector.tensor_copy(ot[:], po[:])
                    eng = nc.sync if m % 2 == 0 else nc.scalar
                    eng.dma_start(out[rows, :], ot[:])


def _build_program():
    nc = bass.Bass(trn_type="TRN2", target_bir_lowering=False, debug=False,
                   num_devices=N_CORES)
    xh = nc.dram_tensor("xh", [HEADS, P, KC, LH], PD, kind="ExternalInput").ap()
    wqkv_t = nc.dram_tensor("wqkv_t", [C, 3 * C], PD, kind="ExternalInput").ap()
    wout_t = nc.dram_tensor("wout_t", [C, C], PD, kind="ExternalInput").ap()
    out = nc.dram_tensor("out", [L, C], PD, kind="ExternalOutput").ap()

    from contextlib import ExitStack
    with tile.TileContext(nc) as tc:
        with ExitStack() as ctx:
            _emit(ctx, tc, xh, wqkv_t, wout_t, out)
    _split_wide_waits(nc)
    return nc


def _host_inputs(x, w_qkv, w_out):
    """Per-core input maps. Permute l so head h owns rows [h*512, (h+1)*512)
    (original row i*8+h -> permuted row h*512+i), then lay x out per-head
    transposed, partition-major: xh[h, p, ko, l] = x_perm[h*512+l, ko*128+p],
    so each per-(h,ko) DMA piece is 128 descriptors of contiguous 2 KiB and
    a whole-head load is 128 descriptors of 8 KiB."""
    import ml_dtypes
    bf16 = ml_dtypes.bfloat16
    wqkv_t = np.ascontiguousarray(w_qkv.T).astype(np.float32).copy()
    wqkv_t[:, 0:C] *= SCALE  # fold the attention scale into the Q weights
    wqkv_t = wqkv_t.astype(bf16)
    wout_t = np.ascontiguousarray(w_out.T).astype(bf16)
    in_maps = []
    for b in range(B):
        xb = x[b]  # (L, C); row l = i*8 + h
        x_perm = xb.reshape(LH, HEADS, C).transpose(1, 0, 2)  # (h, lh, c)
        xh = np.ascontiguousarray(
            x_perm.transpose(0, 2, 1)          # (h, c, lh)
            .reshape(HEADS, KC, P, LH)         # c = ko*128 + p
            .transpose(0, 2, 1, 3)).astype(bf16)  # (h, p, ko, lh)
        in_maps.append({"xh": xh, "wqkv_t": wqkv_t, "wout_t": wout_t})
    return in_maps


def _unpermute(out_perm):
    """(L, C) with rows grouped by head -> original row order i*8+h."""
    return out_perm.reshape(HEADS, LH, C).transpose(1, 0, 2).reshape(L, C)


def kernel(x, w_qkv, w_out, b_out, _run_kwargs=None):
    x = np.asarray(x, dtype=np.float32)
    w_qkv = np.asarray(w_qkv, dtype=np.float32)
    w_out = np.asarray(w_out, dtype=np.float32)
    b_out = np.asarray(b_out, dtype=np.float32)

    nc = _build_program()
    in_maps = _host_inputs(x, w_qkv, w_out)
    res = run_bass_kernel_spmd(nc, in_maps, list(range(N_CORES)),
                               **(_run_kwargs or {}))
    out = np.empty((B, L, C), dtype=np.float32)
    for b in range(B):
        out[b] = _unpermute(res.results[b]["out"].astype(np.float32))
    out += b_out
    if _run_kwargs:
        kernel.last_result = res
    return out


# revision 19
# speedup vs baseline: 1.0028x; 1.0028x over previous
"""Channel-attention transformer block on 8 Trainium2 NeuronCores.

Reference semantics (b=8, l=4096, c=512, h=8 heads carved from the
*sequence* axis, head_pos = l % 8):
    qkv = x @ w_qkv.T ; split q,k,v per head  (each (lh=512, c=512))
    attn = softmax((q.T @ k) / 8, axis=-1)    # (c, c) channel attention
    y.T  = attn @ v.T                         # (c, lh)
    out  = y @ w_out.T + b_out
Sharding: data-parallel over batch — core i handles batch i.

Per-core layout trick: the l axis is permuted on the host so each head's
512 rows are contiguous (row h*512+i <- original row i*8+h), and x is
shipped per-head transposed with per-partition-contiguous blocks
xh[h][p][ko][l] (c = ko*128 + p), so every x DMA is 128 descriptors of
contiguous 8 KiB (vs 512x512B for a plain (c,l) layout). Then per head:
  - Q,K in natural (l, c) layout and V^T in (c, l) layout all come
    straight out of matmuls against xh (no on-device transposes),
  - scores are computed *transposed* (S^T = K^T Q via lhsT=K, rhs=Q) so
    softmax's sum over the attended axis lands on the partition dim,
    where it is computed by a matmul against ones columns glued onto
    V^T (columns 0-1 of the AV rhs) — again no transposes,
  - normalization (multiply by 1/denominator, a per-partition scalar)
    is fused into the PSUM->SBUF evacuation of the AV result,
  - the out-projection consumes y^T (c on partitions) directly as lhsT.
The host un-permutes rows of the returned (4096, 512) per-core output.

Startup choreography (the PE stream is ~99% dense once running, so the
wins are at the edges): DIRECT2D descriptor generation serializes per
issuing queue at ~0.6-1.0 us per dma_start, so the first-needed pieces
(head-0 x split per ko, q/k weight pieces) are interleaved across BOTH
HWDGE queues (sync + scalar) in deadline order, and the QK projection
iterates ko-outer/m-inner (interleaved PSUM accumulation groups) so the
first matmul needs only ONE x piece and ONE weight piece. ~3us of
warmup matmuls on zeros bridge the PE p-state ramp while the first
operands are still in flight. The out-projection is also ko-outer for
heads 0-6 so the matmul consuming the last-normalized y strip has
~2.6us of buffered PE work in front of it (no idle gap -> no p-state
dip). All DMA stays on the two HWDGE queues (SWDGE unused, which drops
its 8 semaphores from the preamble init and the teardown storm);
output stores alternate sync/scalar and the final strip is computed as
two 256-col groups so its first half stores while the second half is
still in the PE.
"""

import numpy as np

import concourse.bass as bass
import concourse.mybir as mybir
import concourse.tile as tile
from concourse.bass_utils import run_bass_kernel_spmd

B = 8
L = 4096
C = 512
HEADS = 8
LH = L // HEADS  # 512
SCALE = 64 ** -0.5  # DIM_HEAD ** -0.5 from the reference
N_CORES = 8
P = 128
KC = C // P  # 4 contraction chunks of 128
F32 = mybir.dt.float32

# Matmul operand dtype: bf16 streams 1 col/cycle through the PE — the same
# throughput as fp32r — but halves every input DMA byte (the startup is
# DMA-ring-bandwidth-bound), halves SBUF traffic, and doubles DVE
# evacuation speed. Accuracy: bf16 rounding is 4x coarser than fp32r's
# TF32, lifting rel err from ~5e-4 to ~2e-3 — far inside the 2e-2 gate.
# PSUM accumulation stays fp32.
MM_DTYPE = mybir.dt.bfloat16
PD = MM_DTYPE  # dtype of every tile that feeds a matmul


def _split_wide_waits(nc, max_waits=1):
    """This container's walrus build rejects instructions carrying more than
    ~1 sync wait ("Too many sync wait commands", e.g. in the S3_LW lowering
    of a fused matmul). Hoist surplus waits onto same-engine nops inserted
    immediately before the offending instruction — the engine stalls at the
    same point in its stream, so scheduling semantics are unchanged."""
    for f in nc.m.functions:
        for bb in f.blocks:
            snapshot = list(bb.instructions)
            if not any(
                inst.sync_info and inst.sync_info.on_wait
                and len(inst.sync_info.on_wait) > max_waits
                for inst in snapshot
            ):
                continue
            new = []
            for inst in snapshot:
                si = inst.sync_info
                waits = list(si.on_wait) if si and si.on_wait else []
                if len(waits) > max_waits:
                    for w in waits[:-max_waits]:
                        nop = nc.engines[inst.engine].nop(nofuse=True).ins
                        cur = nc.cur_bb.bb.instructions
                        assert cur[-1] is nop
                        cur.pop()  # re-homed below, right before `inst`
                        nop.sync_info = mybir.SyncInfo(on_wait=[w], on_update=[])
                        new.append(nop)
                    inst.sync_info = mybir.SyncInfo(
                        on_wait=waits[-max_waits:],
                        on_update=list(si.on_update) if si.on_update else [],
                    )
                new.append(inst)
            bb.instructions = new


def _emit(ctx, tc, xh, wqkv_t, wout_t, out):
    """Emit the per-core program. All DRAM APs:
    xh (HEADS, P, KC, LH) fp32 (per-head transposed x, partition-major),
    wqkv_t (C, 3C) fp32 (q block pre-scaled), wout_t (C, C),
    out (L, C)."""
    nc = tc.nc
    EXP = mybir.ActivationFunctionType.Exp

    xh_r = xh.rearrange("h p ko l -> p h ko l")
    wqkv_r = wqkv_t.rearrange("(ko p) n -> p ko n", p=P)
    wout_r = wout_t.rearrange("(ko p) n -> p ko n", p=P)

    consts = ctx.enter_context(tc.tile_pool(name="consts", bufs=1))
    # bufs=2 (not 3) doubles as a DMA-ring throttle: head h+1's x load
    # acquires the buffer of head h-1, so it can't hit the rings until
    # V-proj(h-1) is done — keeping the startup-critical weight/x pieces
    # from being starved by background prefetch traffic.
    xt_pool = ctx.enter_context(tc.tile_pool(name="xt", bufs=2))
    q_pool = ctx.enter_context(tc.tile_pool(name="q", bufs=3))
    k_pool = ctx.enter_context(tc.tile_pool(name="k", bufs=3))
    vt_pool = ctx.enter_context(tc.tile_pool(name="vt", bufs=3))
    exp_pool = ctx.enter_context(tc.tile_pool(name="exp", bufs=3))
    y_pool = ctx.enter_context(tc.tile_pool(name="y", bufs=3))
    out_pool = ctx.enter_context(tc.tile_pool(name="out", bufs=8))
    recip_pool = ctx.enter_context(tc.tile_pool(name="recip", bufs=8))
    pp_mm = ctx.enter_context(tc.tile_pool(name="pp_mm", bufs=8, space="PSUM"))

    wqkv = consts.tile([P, KC, 3 * C], PD)
    wout = consts.tile([P, KC, C], PD)
    xth0 = xt_pool.tile([P, KC, LH], PD, tag="xth")

    # PE warmup: ~3.2us of small matmuls on zeros, overlapping the DMA
    # lead-in, so the PE p-state ramp (full speed only after ~3us of
    # continuous execution) completes right as the first real operands
    # land (~11.2us). 256-col units keep the warmup->real handoff fine-
    # grained so real work is not queued behind a long warmup op.
    wu = consts.tile([P, 2 * P], PD)
    nc.vector.memset(wu[:], 0.0)
    pwu = pp_mm.tile([P, 2 * P], F32, tag="mm")
    for _ in range(19):
        nc.tensor.matmul(pwu[:], wu[:, 0:P], wu[:], start=True, stop=True)

    def ld_x0(eng, ko):
        eng.dma_start(xth0[:, ko, :], xh_r[:, 0, ko, :])

    def ld_w(eng, j, ko):
        eng.dma_start(wqkv[:, ko, bass.ts(j, C)], wqkv_r[:, ko, bass.ts(j, C)])

    # Deadline-ordered startup pieces, interleaved across the two HWDGE
    # queues. With ko-outer QK groups the PE consumes (x0[ko], wq[ko])
    # pairs every ~0.9 us, matching each queue's ~0.6 us/piece
    # descriptor-generation cadence.
    ld_x0(nc.sync, 0)
    ld_w(nc.sync, 0, 0)
    ld_w(nc.scalar, 0, 1)
    ld_x0(nc.scalar, 1)
    ld_x0(nc.sync, 2)
    ld_w(nc.sync, 0, 2)
    ld_w(nc.scalar, 0, 3)
    ld_x0(nc.scalar, 3)
    ld_w(nc.sync, 1, 0)
    ld_w(nc.scalar, 1, 1)
    ld_w(nc.sync, 1, 2)
    ld_w(nc.scalar, 1, 3)
    ld_w(nc.sync, 2, 0)   # v-block weights, needed from ~T0+7.3us
    ld_w(nc.scalar, 2, 1)
    ld_w(nc.sync, 2, 2)
    ld_w(nc.scalar, 2, 3)
    nc.scalar.dma_start(wout[:], wout_r[:])  # needed from ~T0+22us
    # Head 1's x on the sync queue AFTER every startup-critical piece:
    # queue position defers its ring traffic past the startup crunch
    # (it isn't needed until ~T0+21us).
    xth1 = xt_pool.tile([P, KC, LH], PD, tag="xth")
    nc.sync.dma_start(xth1[:], xh_r[:, 1, :, :])

    for h in range(HEADS):
        if h == 0:
            xth = xth0
        elif h == 1:
            xth = xth1
        else:
            # Alternate the HWDGE queues for the x prefetch: keeps SWDGE
            # (gpsimd) completely unused, which drops the 8 DMASW
            # semaphores from the preamble init and teardown storm. The
            # WAR wait on the xt buffer (V-proj of head h-2) has long
            # released by the time the queue reaches this instruction.
            xth = xt_pool.tile([P, KC, LH], PD, tag="xth")
            eng = nc.sync if h % 2 == 0 else nc.scalar
            eng.dma_start(xth[:], xh_r[:, h, :, :])

        # ---- projections: Q,K natural (l, c); V^T (c, l) with ones col ----
        # ko-outer with 4 interleaved PSUM accumulation groups (one per l'
        # strip m): the first matmul of head 0 depends on just one x piece
        # and one weight piece instead of all four.
        q = q_pool.tile([P, KC, C], PD)
        k = k_pool.tile([P, KC, C], PD)
        for j, dst in ((0, q), (1, k)):
            pqs = [pp_mm.tile([P, C], F32, tag="mm", name=f"pq{j}_{m}")
                   for m in range(KC)]
            for ko in range(KC):
                for m in range(KC):
                    nc.tensor.matmul(
                        pqs[m][:], xth[:, ko, bass.ts(m, P)],
                        wqkv[:, ko, bass.ts(j, C)],
                        start=(ko == 0), stop=(ko == KC - 1))
            for m in range(KC):
                nc.vector.tensor_copy(dst[:, m, :], pqs[m][:])

        vt = vt_pool.tile([P, KC, LH + 2], PD)
        nc.vector.memset(vt[:, :, 0:2], 1.0)
        for m in range(KC):  # c_v strips of 128
            pv = pp_mm.tile([P, LH], F32, tag="mm")
            for ko in range(KC):
                nc.tensor.matmul(
                    pv[:], wqkv[:, ko, bass.ds(2 * C + m * P, P)],
                    xth[:, ko, :],
                    start=(ko == 0), stop=(ko == KC - 1))
            # Evacuate on the scalar (Activation) engine: during the V
            # stage the vector queue already carries the k-stage CASTs
            # (~2.8us) and adding vt would oversubscribe it (5.6us of DVE
            # work in a 3.5us window); scalar idles here.
            nc.scalar.activation(vt[:, m, 2:LH + 2], pv[:],
                                 mybir.ActivationFunctionType.Copy)

        # ---- scores transposed + exp:  S^T[d, c] = sum_l K[l,d] Q[l,c] ----
        ex = exp_pool.tile([P, KC, C], PD)
        for ds_ in range(KC):  # d strips of 128
            ps = pp_mm.tile([P, C], F32, tag="mm")
            for m in range(KC):  # contraction over l' chunks
                nc.tensor.matmul(
                    ps[:], k[:, m, bass.ts(ds_, P)],
                    q[:, m, :],
                    start=(m == 0), stop=(m == KC - 1))
            nc.scalar.activation(ex[:, ds_, :], ps[:], EXP)

        # ---- AV with fused denominator (rhs cols 0,1 are ones; the 514
        # output columns are split 258+256 because a matmul dst cannot
        # exceed one PSUM bank = 512 fp32) ----
        NY1 = 258  # 2 (denominator twice) + 256 v columns
        NY2 = 256
        y = y_pool.tile([P, KC, LH], PD)
        for cs in range(KC):  # c strips of 128
            py1 = pp_mm.tile([P, NY1], F32, tag="mm")
            py2 = pp_mm.tile([P, NY2], F32, tag="mm")
            for ko in range(KC):  # contraction over d chunks
                lhsT = ex[:, ko, bass.ts(cs, P)]
                nc.tensor.matmul(py1[:], lhsT, vt[:, ko, 0:NY1],
                                 start=(ko == 0), stop=(ko == KC - 1))
            for ko in range(KC):
                lhsT = ex[:, ko, bass.ts(cs, P)]
                nc.tensor.matmul(py2[:], lhsT, vt[:, ko, NY1:LH + 2],
                                 start=(ko == 0), stop=(ko == KC - 1))
            rc = recip_pool.tile([P, 1], F32)
            nc.vector.reciprocal(rc[:], py1[:, 0:1])
            nc.vector.tensor_scalar_mul(y[:, cs, 0:NY1 - 2], py1[:, 2:NY1], rc[:])
            nc.vector.tensor_scalar_mul(y[:, cs, NY1 - 2:LH], py2[:], rc[:])

        # ---- out projection: out[l, co] = sum_c y^T[c, l] woutT[c, co] ----
        if h < HEADS - 1:
            pos = [pp_mm.tile([P, C], F32, tag="mm", name=f"po_{m}")
                   for m in range(KC)]
            for ko in range(KC):
                for m in range(KC):
                    nc.tensor.matmul(
                        pos[m][:], y[:, ko, bass.ts(m, P)],
                        wout[:, ko, :],
                        start=(ko == 0), stop=(ko == KC - 1))
            for m in range(KC):
                ot = out_pool.tile([P, C], PD)
                nc.vector.tensor_copy(ot[:], pos[m][:])
                eng = nc.sync if m % 2 == 0 else nc.scalar
                eng.dma_start(out[bass.ds(h * LH + m * P, P), :], ot[:])
        else:
            for m in range(KC):  # l' strips of 128
                rows = bass.ds(h * LH + m * P, P)
                if m == KC - 1:
                    # Final strip: two 256-col accumulation groups so the
                    # first half evacuates + stores while the second half's
                    # matmuls still run; halves go out on both HWDGE queues.
                    pa = pp_mm.tile([P, 384], F32, tag="mm")
                    pb = pp_mm.tile([P, 128], F32, tag="mm")
                    ot = out_pool.tile([P, C], PD)
                    for ko in range(KC):
                        nc.tensor.matmul(
                            pa[:], y[:, ko, bass.ts(m, P)],
                            wout[:, ko, 0:384],
                            start=(ko == 0), stop=(ko == KC - 1))
                    nc.vector.tensor_copy(ot[:, 0:384], pa[:])
                    nc.sync.dma_start(out[rows, 0:384], ot[:, 0:384])
                    for ko in range(KC):
                        nc.tensor.matmul(
                            pb[:], y[:, ko, bass.ts(m, P)],
                            wout[:, ko, 384:512],
                            start=(ko == 0), stop=(ko == KC - 1))
                    nc.vector.tensor_copy(ot[:, 384:512], pb[:])
                    nc.scalar.dma_start(out[rows, 384:512], ot[:, 384:512])
                else:
                    po = pp_mm.tile([P, C], F32, tag="mm")
                    for ko in range(KC):
                        nc.tensor.matmul(
                            po[:], y[:, ko, bass.ts(m, P)],
                            wout[:, ko, :],
                            start=(ko == 0), stop=(ko == KC - 1))
                    ot = out_pool.tile([P, C], PD)
                    nc.vector.tensor_copy(ot[:], po[:])
                    eng = nc.sync if m % 2 == 0 else nc.scalar
                    eng.dma_start(out[rows, :], ot[:])


def _build_program():
    nc = bass.Bass(trn_type="TRN2", target_bir_lowering=False, debug=False,
                   num_devices=N_CORES)
    xh = nc.dram_tensor("xh", [HEADS, P, KC, LH], PD, kind="ExternalInput").ap()
    wqkv_t = nc.dram_tensor("wqkv_t", [C, 3 * C], PD, kind="ExternalInput").ap()
    wout_t = nc.dram_tensor("wout_t", [C, C], PD, kind="ExternalInput").ap()
    out = nc.dram_tensor("out", [L, C], PD, kind="ExternalOutput").ap()

    from contextlib import ExitStack
    with tile.TileContext(nc) as tc:
        with ExitStack() as ctx:
            _emit(ctx, tc, xh, wqkv_t, wout_t, out)
    _split_wide_waits(nc)
    return nc


def _host_inputs(x, w_qkv, w_out):
    """Per-core input maps. Permute l so head h owns rows [h*512, (h+1)*512)
    (original row i*8+h -> permuted row h*512+i), then lay x out per-head
    transposed, partition-major: xh[h, p, ko, l] = x_perm[h*512+l, ko*128+p],
    so each per-(h,ko) DMA piece is 128 descriptors of contiguous 2 KiB and
    a whole-head load is 128 descriptors of 8 KiB."""
    import ml_dtypes
    bf16 = ml_dtypes.bfloat16
    wqkv_t = np.ascontiguousarray(w_qkv.T).astype(np.float32).copy()
    wqkv_t[:, 0:C] *= SCALE  # fold the attention scale into the Q weights
    wqkv_t = wqkv_t.astype(bf16)
    wout_t = np.ascontiguousarray(w_out.T).astype(bf16)
    in_maps = []
    for b in range(B):
        xb = x[b]  # (L, C); row l = i*8 + h
        x_perm = xb.reshape(LH, HEADS, C).transpose(1, 0, 2)  # (h, lh, c)
        xh = np.ascontiguousarray(
            x_perm.transpose(0, 2, 1)          # (h, c, lh)
            .reshape(HEADS, KC, P, LH)         # c = ko*128 + p
            .transpose(0, 2, 1, 3)).astype(bf16)  # (h, p, ko, lh)
        in_maps.append({"xh": xh, "wqkv_t": wqkv_t, "wout_t": wout_t})
    return in_maps


def _unpermute(out_perm):
    """(L, C) with rows grouped by head -> original row order i*8+h."""
    return out_perm.reshape(HEADS, LH, C).transpose(1, 0, 2).reshape(L, C)


def kernel(x, w_qkv, w_out, b_out, _run_kwargs=None):
    x = np.asarray(x, dtype=np.float32)
    w_qkv = np.asarray(w_qkv, dtype=np.float32)
    w_out = np.asarray(w_out, dtype=np.float32)
    b_out = np.asarray(b_out, dtype=np.float32)

    nc = _build_program()
    in_maps = _host_inputs(x, w_qkv, w_out)
    res = run_bass_kernel_spmd(nc, in_maps, list(range(N_CORES)),
                               **(_run_kwargs or {}))
    out = np.empty((B, L, C), dtype=np.float32)
    for b in range(B):
        out[b] = _unpermute(res.results[b]["out"].astype(np.float32))
    out += b_out
    if _run_kwargs:
        kernel.last_result = res
    return out


# revision 20
# speedup vs baseline: 1.1938x; 1.1905x over previous
"""Channel-attention transformer block on 8 Trainium2 NeuronCores.

Reference semantics (b=8, l=4096, c=512, h=8 heads carved from the
*sequence* axis, head_pos = l % 8):
    qkv = x @ w_qkv.T ; split q,k,v per head  (each (lh=512, c=512))
    attn = softmax((q.T @ k) / 8, axis=-1)    # (c, c) channel attention
    y.T  = attn @ v.T                         # (c, lh)
    out  = y @ w_out.T + b_out
Sharding: data-parallel over batch — core i handles batch i.

Per-core layout trick: the l axis is permuted on the host so each head's
512 rows are contiguous (row h*512+i <- original row i*8+h), and x is
shipped per-head transposed with per-partition-contiguous blocks
xh[h][p][ko][l] (c = ko*128 + p), so every x DMA is 128 descriptors of
contiguous 8 KiB (vs 512x512B for a plain (c,l) layout). Then per head:
  - Q,K in natural (l, c) layout and V^T in (c, l) layout all come
    straight out of matmuls against xh (no on-device transposes),
  - scores are computed *transposed* (S^T = K^T Q via lhsT=K, rhs=Q) so
    softmax's sum over the attended axis lands on the partition dim,
    where it is computed by a matmul against ones columns glued onto
    V^T (columns 0-1 of the AV rhs) — again no transposes,
  - normalization (multiply by 1/denominator, a per-partition scalar)
    is fused into the PSUM->SBUF evacuation of the AV result,
  - the out-projection consumes y^T (c on partitions) directly as lhsT.
The host un-permutes rows of the returned (4096, 512) per-core output.

Startup choreography (the PE stream is ~99% dense once running, so the
wins are at the edges): DIRECT2D descriptor generation serializes per
issuing queue at ~0.6-1.0 us per dma_start, so the first-needed pieces
(head-0 x split per ko, q/k weight pieces) are interleaved across BOTH
HWDGE queues (sync + scalar) in deadline order, and the QK projection
iterates ko-outer/m-inner (interleaved PSUM accumulation groups) so the
first matmul needs only ONE x piece and ONE weight piece. ~3us of
warmup matmuls on zeros bridge the PE p-state ramp while the first
operands are still in flight. The out-projection is also ko-outer for
heads 0-6 so the matmul consuming the last-normalized y strip has
~2.6us of buffered PE work in front of it (no idle gap -> no p-state
dip). All DMA stays on the two HWDGE queues (SWDGE unused, which drops
its 8 semaphores from the preamble init and the teardown storm);
output stores alternate sync/scalar and the final strip is computed as
two 256-col groups so its first half stores while the second half is
still in the PE.
"""

import numpy as np

import concourse.bass as bass
import concourse.mybir as mybir
import concourse.tile as tile
from concourse.bass_utils import run_bass_kernel_spmd

B = 8
L = 4096
C = 512
HEADS = 8
LH = L // HEADS  # 512
SCALE = 64 ** -0.5  # DIM_HEAD ** -0.5 from the reference
N_CORES = 8
P = 128
KC = C // P  # 4 contraction chunks of 128
F32 = mybir.dt.float32

# Matmul operand dtype: bf16 streams 1 col/cycle through the PE — the same
# throughput as fp32r — but halves every input DMA byte (the startup is
# DMA-ring-bandwidth-bound), halves SBUF traffic, and doubles DVE
# evacuation speed. Accuracy: bf16 rounding is 4x coarser than fp32r's
# TF32, lifting rel err from ~5e-4 to ~2e-3 — far inside the 2e-2 gate.
# PSUM accumulation stays fp32.
MM_DTYPE = mybir.dt.bfloat16
PD = MM_DTYPE  # dtype of every tile that feeds a matmul


def _split_wide_waits(nc, max_waits=1):
    """This container's walrus build rejects instructions carrying more than
    ~1 sync wait ("Too many sync wait commands", e.g. in the S3_LW lowering
    of a fused matmul). Hoist surplus waits onto same-engine nops inserted
    immediately before the offending instruction — the engine stalls at the
    same point in its stream, so scheduling semantics are unchanged."""
    for f in nc.m.functions:
        for bb in f.blocks:
            snapshot = list(bb.instructions)
            if not any(
                inst.sync_info and inst.sync_info.on_wait
                and len(inst.sync_info.on_wait) > max_waits
                for inst in snapshot
            ):
                continue
            new = []
            for inst in snapshot:
                si = inst.sync_info
                waits = list(si.on_wait) if si and si.on_wait else []
                if len(waits) > max_waits:
                    for w in waits[:-max_waits]:
                        nop = nc.engines[inst.engine].nop(nofuse=True).ins
                        cur = nc.cur_bb.bb.instructions
                        assert cur[-1] is nop
                        cur.pop()  # re-homed below, right before `inst`
                        nop.sync_info = mybir.SyncInfo(on_wait=[w], on_update=[])
                        new.append(nop)
                    inst.sync_info = mybir.SyncInfo(
                        on_wait=waits[-max_waits:],
                        on_update=list(si.on_update) if si.on_update else [],
                    )
                new.append(inst)
            bb.instructions = new


def _emit(ctx, tc, xh, wqkv_t, wout_t, out):
    """Emit the per-core program. All DRAM APs:
    xh (HEADS, P, KC, LH) fp32 (per-head transposed x, partition-major),
    wqkv_t (C, 3C) fp32 (q block pre-scaled), wout_t (C, C),
    out (L, C)."""
    nc = tc.nc
    EXP = mybir.ActivationFunctionType.Exp

    xh_r = xh.rearrange("h p ko l -> p h ko l")
    wqkv_r = wqkv_t.rearrange("(ko p) n -> p ko n", p=P)
    wout_r = wout_t.rearrange("(ko p) n -> p ko n", p=P)

    consts = ctx.enter_context(tc.tile_pool(name="consts", bufs=1))
    # bufs=2 (not 3) doubles as a DMA-ring throttle: head h+1's x load
    # acquires the buffer of head h-1, so it can't hit the rings until
    # V-proj(h-1) is done — keeping the startup-critical weight/x pieces
    # from being starved by background prefetch traffic.
    xt_pool = ctx.enter_context(tc.tile_pool(name="xt", bufs=2))
    q_pool = ctx.enter_context(tc.tile_pool(name="q", bufs=3))
    k_pool = ctx.enter_context(tc.tile_pool(name="k", bufs=3))
    vt_pool = ctx.enter_context(tc.tile_pool(name="vt", bufs=3))
    exp_pool = ctx.enter_context(tc.tile_pool(name="exp", bufs=3))
    y_pool = ctx.enter_context(tc.tile_pool(name="y", bufs=3))
    out_pool = ctx.enter_context(tc.tile_pool(name="out", bufs=8))
    recip_pool = ctx.enter_context(tc.tile_pool(name="recip", bufs=8))
    pp_mm = ctx.enter_context(tc.tile_pool(name="pp_mm", bufs=8, space="PSUM"))

    wqkv = consts.tile([P, KC, 3 * C], PD)
    wout = consts.tile([P, KC, C], PD)
    xth0 = xt_pool.tile([P, KC, LH], PD, tag="xth")

    # PE warmup: ~3.2us of small matmuls on zeros, overlapping the DMA
    # lead-in, so the PE p-state ramp (full speed only after ~3us of
    # continuous execution) completes right as the first real operands
    # land (~11.2us). 256-col units keep the warmup->real handoff fine-
    # grained so real work is not queued behind a long warmup op.
    wu = consts.tile([P, 2 * P], PD)
    nc.vector.memset(wu[:], 0.0)
    pwu = pp_mm.tile([P, 2 * P], F32, tag="mm")
    for _ in range(19):
        nc.tensor.matmul(pwu[:], wu[:, 0:P], wu[:], start=True, stop=True)

    def ld_x0(eng, ko):
        eng.dma_start(xth0[:, ko, :], xh_r[:, 0, ko, :])

    def ld_w(eng, j, ko):
        eng.dma_start(wqkv[:, ko, bass.ts(j, C)], wqkv_r[:, ko, bass.ts(j, C)])

    # Deadline-ordered startup pieces, interleaved across the two HWDGE
    # queues. With ko-outer QK groups the PE consumes (x0[ko], wq[ko])
    # pairs every ~0.9 us, matching each queue's ~0.6 us/piece
    # descriptor-generation cadence.
    ld_x0(nc.sync, 0)
    ld_w(nc.sync, 0, 0)
    ld_w(nc.scalar, 0, 1)
    ld_x0(nc.scalar, 1)
    ld_x0(nc.sync, 2)
    ld_w(nc.sync, 0, 2)
    ld_w(nc.scalar, 0, 3)
    ld_x0(nc.scalar, 3)
    ld_w(nc.sync, 1, 0)
    ld_w(nc.scalar, 1, 1)
    ld_w(nc.sync, 1, 2)
    ld_w(nc.scalar, 1, 3)
    ld_w(nc.sync, 2, 0)   # v-block weights, needed from ~T0+7.3us
    ld_w(nc.scalar, 2, 1)
    ld_w(nc.sync, 2, 2)
    ld_w(nc.scalar, 2, 3)
    nc.scalar.dma_start(wout[:], wout_r[:])  # needed from ~T0+22us
    # Head 1's x on the sync queue AFTER every startup-critical piece:
    # queue position defers its ring traffic past the startup crunch
    # (it isn't needed until ~T0+21us).
    xth1 = xt_pool.tile([P, KC, LH], PD, tag="xth")
    nc.sync.dma_start(xth1[:], xh_r[:, 1, :, :])

    for h in range(HEADS):
        if h == 0:
            xth = xth0
        elif h == 1:
            xth = xth1
        else:
            # Alternate the HWDGE queues for the x prefetch: keeps SWDGE
            # (gpsimd) completely unused, which drops the 8 DMASW
            # semaphores from the preamble init and teardown storm. The
            # WAR wait on the xt buffer (V-proj of head h-2) has long
            # released by the time the queue reaches this instruction.
            xth = xt_pool.tile([P, KC, LH], PD, tag="xth")
            eng = nc.sync if h % 2 == 0 else nc.scalar
            eng.dma_start(xth[:], xh_r[:, h, :, :])

        # ---- projections: Q,K natural (l, c); V^T (c, l) with ones col ----
        # ko-outer with 4 interleaved PSUM accumulation groups (one per l'
        # strip m): the first matmul of head 0 depends on just one x piece
        # and one weight piece instead of all four.
        q = q_pool.tile([P, KC, C], PD)
        k = k_pool.tile([P, KC, C], PD)
        for j, dst in ((0, q), (1, k)):
            pqs = [pp_mm.tile([P, C], F32, tag="mm", name=f"pq{j}_{m}")
                   for m in range(KC)]
            for ko in range(KC):
                for m in range(KC):
                    nc.tensor.matmul(
                        pqs[m][:], xth[:, ko, bass.ts(m, P)],
                        wqkv[:, ko, bass.ts(j, C)],
                        start=(ko == 0), stop=(ko == KC - 1))
            for m in range(KC):
                nc.vector.tensor_copy(dst[:, m, :], pqs[m][:])

        vt = vt_pool.tile([P, KC, LH + 2], PD)
        nc.vector.memset(vt[:, :, 0:2], 1.0)
        for m in range(KC):  # c_v strips of 128
            pv = pp_mm.tile([P, LH], F32, tag="mm")
            for ko in range(KC):
                nc.tensor.matmul(
                    pv[:], wqkv[:, ko, bass.ds(2 * C + m * P, P)],
                    xth[:, ko, :],
                    start=(ko == 0), stop=(ko == KC - 1))
            # Evacuate on the scalar (Activation) engine: during the V
            # stage the vector queue already carries the k-stage CASTs
            # (~2.8us) and adding vt would oversubscribe it (5.6us of DVE
            # work in a 3.5us window); scalar idles here.
            nc.scalar.activation(vt[:, m, 2:LH + 2], pv[:],
                                 mybir.ActivationFunctionType.Copy)

        # ---- scores transposed + exp:  S^T[d, c] = sum_l K[l,d] Q[l,c] ----
        ex = exp_pool.tile([P, KC, C], PD)
        for ds_ in range(KC):  # d strips of 128
            ps = pp_mm.tile([P, C], F32, tag="mm")
            for m in range(KC):  # contraction over l' chunks
                nc.tensor.matmul(
                    ps[:], k[:, m, bass.ts(ds_, P)],
                    q[:, m, :],
                    start=(m == 0), stop=(m == KC - 1))
            nc.scalar.activation(ex[:, ds_, :], ps[:], EXP)

        # ---- AV with fused denominator (rhs cols 0,1 are ones; the 514
        # output columns are split 258+256 because a matmul dst cannot
        # exceed one PSUM bank = 512 fp32) ----
        NY1 = 258  # 2 (denominator twice) + 256 v columns
        NY2 = 256
        y = y_pool.tile([P, KC, LH], PD)
        for cs in range(KC):  # c strips of 128
            py1 = pp_mm.tile([P, NY1], F32, tag="mm")
            py2 = pp_mm.tile([P, NY2], F32, tag="mm")
            for ko in range(KC):  # contraction over d chunks
                lhsT = ex[:, ko, bass.ts(cs, P)]
                nc.tensor.matmul(py1[:], lhsT, vt[:, ko, 0:NY1],
                                 start=(ko == 0), stop=(ko == KC - 1))
            for ko in range(KC):
                lhsT = ex[:, ko, bass.ts(cs, P)]
                nc.tensor.matmul(py2[:], lhsT, vt[:, ko, NY1:LH + 2],
                                 start=(ko == 0), stop=(ko == KC - 1))
            rc = recip_pool.tile([P, 1], F32)
            nc.vector.reciprocal(rc[:], py1[:, 0:1])
            nc.vector.tensor_scalar_mul(y[:, cs, 0:NY1 - 2], py1[:, 2:NY1], rc[:])
            nc.vector.tensor_scalar_mul(y[:, cs, NY1 - 2:LH], py2[:], rc[:])

        # ---- out projection: out[l, co] = sum_c y^T[c, l] woutT[c, co] ----
        if h < HEADS - 1:
            pos = [pp_mm.tile([P, C], F32, tag="mm", name=f"po_{m}")
                   for m in range(KC)]
            for ko in range(KC):
                for m in range(KC):
                    nc.tensor.matmul(
                        pos[m][:], y[:, ko, bass.ts(m, P)],
                        wout[:, ko, :],
                        start=(ko == 0), stop=(ko == KC - 1))
            for m in range(KC):
                # Evacuate on scalar: balances DVE load (vector otherwise
                # carries ~69% of a head window) and makes the m1/m3
                # stores same-queue with their copies (no cross-engine
                # semaphore hop).
                ot = out_pool.tile([P, C], PD)
                nc.scalar.activation(ot[:], pos[m][:],
                                     mybir.ActivationFunctionType.Copy)
                eng = nc.sync if m % 2 == 0 else nc.scalar
                eng.dma_start(out[bass.ds(h * LH + m * P, P), :], ot[:])
        else:
            for m in range(KC):  # l' strips of 128
                rows = bass.ds(h * LH + m * P, P)
                if m == KC - 1:
                    # Final strip: two 256-col accumulation groups so the
                    # first half evacuates + stores while the second half's
                    # matmuls still run; halves go out on both HWDGE queues.
                    pa = pp_mm.tile([P, 384], F32, tag="mm")
                    pb = pp_mm.tile([P, 128], F32, tag="mm")
                    ot = out_pool.tile([P, C], PD)
                    for ko in range(KC):
                        nc.tensor.matmul(
                            pa[:], y[:, ko, bass.ts(m, P)],
                            wout[:, ko, 0:384],
                            start=(ko == 0), stop=(ko == KC - 1))
                    nc.vector.tensor_copy(ot[:, 0:384], pa[:])
                    nc.sync.dma_start(out[rows, 0:384], ot[:, 0:384])
                    for ko in range(KC):
                        nc.tensor.matmul(
                            pb[:], y[:, ko, bass.ts(m, P)],
                            wout[:, ko, 384:512],
                            start=(ko == 0), stop=(ko == KC - 1))
                    nc.vector.tensor_copy(ot[:, 384:512], pb[:])
                    nc.scalar.dma_start(out[rows, 384:512], ot[:, 384:512])
                else:
                    po = pp_mm.tile([P, C], F32, tag="mm")
                    for ko in range(KC):
                        nc.tensor.matmul(
                            po[:], y[:, ko, bass.ts(m, P)],
                            wout[:, ko, :],
                            start=(ko == 0), stop=(ko == KC - 1))
                    ot = out_pool.tile([P, C], PD)
                    nc.vector.tensor_copy(ot[:], po[:])
                    eng = nc.sync if m % 2 == 0 else nc.scalar
                    eng.dma_start(out[rows, :], ot[:])


def _build_program():
    nc = bass.Bass(trn_type="TRN2", target_bir_lowering=False, debug=False,
                   num_devices=N_CORES)
    xh = nc.dram_tensor("xh", [HEADS, P, KC, LH], PD, kind="ExternalInput").ap()
    wqkv_t = nc.dram_tensor("wqkv_t", [C, 3 * C], PD, kind="ExternalInput").ap()
    wout_t = nc.dram_tensor("wout_t", [C, C], PD, kind="ExternalInput").ap()
    out = nc.dram_tensor("out", [L, C], PD, kind="ExternalOutput").ap()

    from contextlib import ExitStack
    with tile.TileContext(nc) as tc:
        with ExitStack() as ctx:
            _emit(ctx, tc, xh, wqkv_t, wout_t, out)
    _split_wide_waits(nc)
    return nc


def _host_inputs(x, w_qkv, w_out):
    """Per-core input maps. Permute l so head h owns rows [h*512, (h+1)*512)
    (original row i*8+h -> permuted row h*512+i), then lay x out per-head
    transposed, partition-major: xh[h, p, ko, l] = x_perm[h*512+l, ko*128+p],
    so each per-(h,ko) DMA piece is 128 descriptors of contiguous 2 KiB and
    a whole-head load is 128 descriptors of 8 KiB."""
    import ml_dtypes
    bf16 = ml_dtypes.bfloat16
    wqkv_t = np.ascontiguousarray(w_qkv.T).astype(np.float32).copy()
    wqkv_t[:, 0:C] *= SCALE  # fold the attention scale into the Q weights
    wqkv_t = wqkv_t.astype(bf16)
    wout_t = np.ascontiguousarray(w_out.T).astype(bf16)
    in_maps = []
    for b in range(B):
        xb = x[b]  # (L, C); row l = i*8 + h
        x_perm = xb.reshape(LH, HEADS, C).transpose(1, 0, 2)  # (h, lh, c)
        xh = np.ascontiguousarray(
            x_perm.transpose(0, 2, 1)          # (h, c, lh)
            .reshape(HEADS, KC, P, LH)         # c = ko*128 + p
            .transpose(0, 2, 1, 3)).astype(bf16)  # (h, p, ko, lh)
        in_maps.append({"xh": xh, "wqkv_t": wqkv_t, "wout_t": wout_t})
    return in_maps


def _unpermute(out_perm):
    """(L, C) with rows grouped by head -> original row order i*8+h."""
    return out_perm.reshape(HEADS, LH, C).transpose(1, 0, 2).reshape(L, C)


def kernel(x, w_qkv, w_out, b_out, _run_kwargs=None):
    x = np.asarray(x, dtype=np.float32)
    w_qkv = np.asarray(w_qkv, dtype=np.float32)
    w_out = np.asarray(w_out, dtype=np.float32)
    b_out = np.asarray(b_out, dtype=np.float32)

    nc = _build_program()
    in_maps = _host_inputs(x, w_qkv, w_out)
    res = run_bass_kernel_spmd(nc, in_maps, list(range(N_CORES)),
                               **(_run_kwargs or {}))
    out = np.empty((B, L, C), dtype=np.float32)
    for b in range(B):
        out[b] = _unpermute(res.results[b]["out"].astype(np.float32))
    out += b_out
    if _run_kwargs:
        kernel.last_result = res
    return out


# revision 21
# speedup vs baseline: 1.2027x; 1.0075x over previous
"""Channel-attention transformer block on 8 Trainium2 NeuronCores.

Reference semantics (b=8, l=4096, c=512, h=8 heads carved from the
*sequence* axis, head_pos = l % 8):
    qkv = x @ w_qkv.T ; split q,k,v per head  (each (lh=512, c=512))
    attn = softmax((q.T @ k) / 8, axis=-1)    # (c, c) channel attention
    y.T  = attn @ v.T                         # (c, lh)
    out  = y @ w_out.T + b_out
Sharding: data-parallel over batch — core i handles batch i.

Per-core layout trick: the l axis is permuted on the host so each head's
512 rows are contiguous (row h*512+i <- original row i*8+h), and x is
shipped per-head transposed with per-partition-contiguous blocks
xh[h][p][ko][l] (c = ko*128 + p), so every x DMA is 128 descriptors of
contiguous 8 KiB (vs 512x512B for a plain (c,l) layout). Then per head:
  - Q,K in natural (l, c) layout and V^T in (c, l) layout all come
    straight out of matmuls against xh (no on-device transposes),
  - scores are computed *transposed* (S^T = K^T Q via lhsT=K, rhs=Q) so
    softmax's sum over the attended axis lands on the partition dim,
    where it is computed by a matmul against ones columns glued onto
    V^T (columns 0-1 of the AV rhs) — again no transposes,
  - normalization (multiply by 1/denominator, a per-partition scalar)
    is fused into the PSUM->SBUF evacuation of the AV result,
  - the out-projection consumes y^T (c on partitions) directly as lhsT.
The host un-permutes rows of the returned (4096, 512) per-core output.

Startup choreography (the PE stream is ~99% dense once running, so the
wins are at the edges): DIRECT2D descriptor generation serializes per
issuing queue at ~0.6-1.0 us per dma_start, so the first-needed pieces
(head-0 x split per ko, q/k weight pieces) are interleaved across BOTH
HWDGE queues (sync + scalar) in deadline order, and the QK projection
iterates ko-outer/m-inner (interleaved PSUM accumulation groups) so the
first matmul needs only ONE x piece and ONE weight piece. ~3us of
warmup matmuls on zeros bridge the PE p-state ramp while the first
operands are still in flight. The out-projection is also ko-outer for
heads 0-6 so the matmul consuming the last-normalized y strip has
~2.6us of buffered PE work in front of it (no idle gap -> no p-state
dip). All DMA stays on the two HWDGE queues (SWDGE unused, which drops
its 8 semaphores from the preamble init and the teardown storm);
output stores alternate sync/scalar and the final strip is computed as
two 256-col groups so its first half stores while the second half is
still in the PE.
"""

import numpy as np

import concourse.bass as bass
import concourse.mybir as mybir
import concourse.tile as tile
from concourse.bass_utils import run_bass_kernel_spmd

B = 8
L = 4096
C = 512
HEADS = 8
LH = L // HEADS  # 512
SCALE = 64 ** -0.5  # DIM_HEAD ** -0.5 from the reference
N_CORES = 8
P = 128
KC = C // P  # 4 contraction chunks of 128
F32 = mybir.dt.float32

# Matmul operand dtype: bf16 streams 1 col/cycle through the PE — the same
# throughput as fp32r — but halves every input DMA byte (the startup is
# DMA-ring-bandwidth-bound), halves SBUF traffic, and doubles DVE
# evacuation speed. Accuracy: bf16 rounding is 4x coarser than fp32r's
# TF32, lifting rel err from ~5e-4 to ~2e-3 — far inside the 2e-2 gate.
# PSUM accumulation stays fp32.
MM_DTYPE = mybir.dt.bfloat16
PD = MM_DTYPE  # dtype of every tile that feeds a matmul


def _split_wide_waits(nc, max_waits=1):
    """This container's walrus build rejects instructions carrying more than
    ~1 sync wait ("Too many sync wait commands", e.g. in the S3_LW lowering
    of a fused matmul). Hoist surplus waits onto same-engine nops inserted
    immediately before the offending instruction — the engine stalls at the
    same point in its stream, so scheduling semantics are unchanged."""
    for f in nc.m.functions:
        for bb in f.blocks:
            snapshot = list(bb.instructions)
            if not any(
                inst.sync_info and inst.sync_info.on_wait
                and len(inst.sync_info.on_wait) > max_waits
                for inst in snapshot
            ):
                continue
            new = []
            for inst in snapshot:
                si = inst.sync_info
                waits = list(si.on_wait) if si and si.on_wait else []
                if len(waits) > max_waits:
                    for w in waits[:-max_waits]:
                        nop = nc.engines[inst.engine].nop(nofuse=True).ins
                        cur = nc.cur_bb.bb.instructions
                        assert cur[-1] is nop
                        cur.pop()  # re-homed below, right before `inst`
                        nop.sync_info = mybir.SyncInfo(on_wait=[w], on_update=[])
                        new.append(nop)
                    inst.sync_info = mybir.SyncInfo(
                        on_wait=waits[-max_waits:],
                        on_update=list(si.on_update) if si.on_update else [],
                    )
                new.append(inst)
            bb.instructions = new


def _emit(ctx, tc, xh, wqkv_t, wout_t, out):
    """Emit the per-core program. All DRAM APs:
    xh (HEADS, P, KC, LH) fp32 (per-head transposed x, partition-major),
    wqkv_t (C, 3C) fp32 (q block pre-scaled), wout_t (C, C),
    out (L, C)."""
    nc = tc.nc
    EXP = mybir.ActivationFunctionType.Exp

    xh_r = xh.rearrange("h p ko l -> p h ko l")
    wqkv_r = wqkv_t.rearrange("(ko p) n -> p ko n", p=P)
    wout_r = wout_t.rearrange("(ko p) n -> p ko n", p=P)

    consts = ctx.enter_context(tc.tile_pool(name="consts", bufs=1))
    # bufs=2 (not 3) doubles as a DMA-ring throttle: head h+1's x load
    # acquires the buffer of head h-1, so it can't hit the rings until
    # V-proj(h-1) is done — keeping the startup-critical weight/x pieces
    # from being starved by background prefetch traffic.
    xt_pool = ctx.enter_context(tc.tile_pool(name="xt", bufs=2))
    q_pool = ctx.enter_context(tc.tile_pool(name="q", bufs=3))
    k_pool = ctx.enter_context(tc.tile_pool(name="k", bufs=3))
    vt_pool = ctx.enter_context(tc.tile_pool(name="vt", bufs=3))
    exp_pool = ctx.enter_context(tc.tile_pool(name="exp", bufs=3))
    y_pool = ctx.enter_context(tc.tile_pool(name="y", bufs=3))
    out_pool = ctx.enter_context(tc.tile_pool(name="out", bufs=8))
    recip_pool = ctx.enter_context(tc.tile_pool(name="recip", bufs=8))
    pp_mm = ctx.enter_context(tc.tile_pool(name="pp_mm", bufs=8, space="PSUM"))

    wqkv = consts.tile([P, KC, 3 * C], PD)
    wout = consts.tile([P, KC, C], PD)
    xth0 = xt_pool.tile([P, KC, LH], PD, tag="xth")

    # PE warmup: ~3.2us of small matmuls on zeros, overlapping the DMA
    # lead-in, so the PE p-state ramp (full speed only after ~3us of
    # continuous execution) completes right as the first real operands
    # land (~11.2us). 256-col units keep the warmup->real handoff fine-
    # grained so real work is not queued behind a long warmup op.
    wu = consts.tile([P, 2 * P], PD)
    nc.vector.memset(wu[:], 0.0)
    pwu = pp_mm.tile([P, 2 * P], F32, tag="mm")
    for _ in range(19):
        nc.tensor.matmul(pwu[:], wu[:, 0:P], wu[:], start=True, stop=True)

    def ld_x0(eng, ko):
        eng.dma_start(xth0[:, ko, :], xh_r[:, 0, ko, :])

    def ld_w(eng, j, ko):
        eng.dma_start(wqkv[:, ko, bass.ts(j, C)], wqkv_r[:, ko, bass.ts(j, C)])

    # Deadline-ordered startup pieces, interleaved across the two HWDGE
    # queues. With ko-outer QK groups the PE consumes (x0[ko], wq[ko])
    # pairs every ~0.9 us, matching each queue's ~0.6 us/piece
    # descriptor-generation cadence.
    ld_x0(nc.sync, 0)
    ld_w(nc.sync, 0, 0)
    ld_w(nc.scalar, 0, 1)
    ld_x0(nc.scalar, 1)
    ld_x0(nc.sync, 2)
    ld_w(nc.sync, 0, 2)
    ld_w(nc.scalar, 0, 3)
    ld_x0(nc.scalar, 3)
    ld_w(nc.sync, 1, 0)
    ld_w(nc.scalar, 1, 1)
    ld_w(nc.sync, 1, 2)
    ld_w(nc.scalar, 1, 3)
    ld_w(nc.sync, 2, 0)   # v-block weights, needed from ~T0+7.3us
    ld_w(nc.scalar, 2, 1)
    ld_w(nc.sync, 2, 2)
    ld_w(nc.scalar, 2, 3)
    nc.scalar.dma_start(wout[:], wout_r[:])  # needed from ~T0+22us
    # Head 1's x on the sync queue AFTER every startup-critical piece:
    # queue position defers its ring traffic past the startup crunch
    # (it isn't needed until ~T0+21us).
    xth1 = xt_pool.tile([P, KC, LH], PD, tag="xth")
    nc.sync.dma_start(xth1[:], xh_r[:, 1, :, :])

    for h in range(HEADS):
        if h == 0:
            xth = xth0
        elif h == 1:
            xth = xth1
        else:
            # Alternate the HWDGE queues for the x prefetch: keeps SWDGE
            # (gpsimd) completely unused, which drops the 8 DMASW
            # semaphores from the preamble init and teardown storm. The
            # WAR wait on the xt buffer (V-proj of head h-2) has long
            # released by the time the queue reaches this instruction.
            xth = xt_pool.tile([P, KC, LH], PD, tag="xth")
            eng = nc.sync if h % 2 == 0 else nc.scalar
            eng.dma_start(xth[:], xh_r[:, h, :, :])

        # ---- projections: Q,K natural (l, c); V^T (c, l) with ones col ----
        # ko-outer with 4 interleaved PSUM accumulation groups (one per l'
        # strip m): the first matmul of head 0 depends on just one x piece
        # and one weight piece instead of all four.
        q = q_pool.tile([P, KC, C], PD)
        k = k_pool.tile([P, KC, C], PD)
        for j, dst in ((0, q), (1, k)):
            pqs = [pp_mm.tile([P, C], F32, tag="mm", name=f"pq{j}_{m}")
                   for m in range(KC)]
            for ko in range(KC):
                for m in range(KC):
                    nc.tensor.matmul(
                        pqs[m][:], xth[:, ko, bass.ts(m, P)],
                        wqkv[:, ko, bass.ts(j, C)],
                        start=(ko == 0), stop=(ko == KC - 1))
            for m in range(KC):
                nc.vector.tensor_copy(dst[:, m, :], pqs[m][:])

        vt = vt_pool.tile([P, KC, LH + 2], PD)
        nc.vector.memset(vt[:, :, 0:2], 1.0)
        for m in range(KC):  # c_v strips of 128
            pv = pp_mm.tile([P, LH], F32, tag="mm")
            for ko in range(KC):
                nc.tensor.matmul(
                    pv[:], wqkv[:, ko, bass.ds(2 * C + m * P, P)],
                    xth[:, ko, :],
                    start=(ko == 0), stop=(ko == KC - 1))
            # Evacuate on the scalar (Activation) engine: during the V
            # stage the vector queue already carries the k-stage CASTs
            # (~2.8us) and adding vt would oversubscribe it (5.6us of DVE
            # work in a 3.5us window); scalar idles here.
            nc.scalar.activation(vt[:, m, 2:LH + 2], pv[:],
                                 mybir.ActivationFunctionType.Copy)

        # ---- scores transposed + exp:  S^T[d, c] = sum_l K[l,d] Q[l,c] ----
        ex = exp_pool.tile([P, KC, C], PD)
        for ds_ in range(KC):  # d strips of 128
            ps = pp_mm.tile([P, C], F32, tag="mm")
            for m in range(KC):  # contraction over l' chunks
                nc.tensor.matmul(
                    ps[:], k[:, m, bass.ts(ds_, P)],
                    q[:, m, :],
                    start=(m == 0), stop=(m == KC - 1))
            nc.scalar.activation(ex[:, ds_, :], ps[:], EXP)

        # ---- AV with fused denominator (rhs cols 0,1 are ones; the 514
        # output columns are split 258+256 because a matmul dst cannot
        # exceed one PSUM bank = 512 fp32) ----
        NY1 = 258  # 2 (denominator twice) + 256 v columns
        NY2 = 256
        y = y_pool.tile([P, KC, LH], PD)
        for cs in range(KC):  # c strips of 128
            py1 = pp_mm.tile([P, NY1], F32, tag="mm")
            py2 = pp_mm.tile([P, NY2], F32, tag="mm")
            for ko in range(KC):  # contraction over d chunks
                lhsT = ex[:, ko, bass.ts(cs, P)]
                nc.tensor.matmul(py1[:], lhsT, vt[:, ko, 0:NY1],
                                 start=(ko == 0), stop=(ko == KC - 1))
            for ko in range(KC):
                lhsT = ex[:, ko, bass.ts(cs, P)]
                nc.tensor.matmul(py2[:], lhsT, vt[:, ko, NY1:LH + 2],
                                 start=(ko == 0), stop=(ko == KC - 1))
            rc = recip_pool.tile([P, 1], F32)
            nc.vector.reciprocal(rc[:], py1[:, 0:1])
            nc.vector.tensor_scalar_mul(y[:, cs, 0:NY1 - 2], py1[:, 2:NY1], rc[:])
            nc.vector.tensor_scalar_mul(y[:, cs, NY1 - 2:LH], py2[:], rc[:])

        # ---- out projection: out[l, co] = sum_c y^T[c, l] woutT[c, co] ----
        if h < HEADS - 1:
            pos = [pp_mm.tile([P, C], F32, tag="mm", name=f"po_{m}")
                   for m in range(KC)]
            for ko in range(KC):
                for m in range(KC):
                    nc.tensor.matmul(
                        pos[m][:], y[:, ko, bass.ts(m, P)],
                        wout[:, ko, :],
                        start=(ko == 0), stop=(ko == KC - 1))
            for m in range(KC):
                ot = out_pool.tile([P, C], PD)
                nc.vector.tensor_copy(ot[:], pos[m][:])
                eng = nc.sync if m % 2 == 0 else nc.scalar
                eng.dma_start(out[bass.ds(h * LH + m * P, P), :], ot[:])
        else:
            for m in range(KC):  # l' strips of 128
                rows = bass.ds(h * LH + m * P, P)
                if m == KC - 1:
                    # Final strip: two 256-col accumulation groups so the
                    # first half evacuates + stores while the second half's
                    # matmuls still run; halves go out on both HWDGE queues.
                    pa = pp_mm.tile([P, 384], F32, tag="mm")
                    pb = pp_mm.tile([P, 128], F32, tag="mm")
                    ot = out_pool.tile([P, C], PD)
                    for ko in range(KC):
                        nc.tensor.matmul(
                            pa[:], y[:, ko, bass.ts(m, P)],
                            wout[:, ko, 0:384],
                            start=(ko == 0), stop=(ko == KC - 1))
                    nc.vector.tensor_copy(ot[:, 0:384], pa[:])
                    nc.sync.dma_start(out[rows, 0:384], ot[:, 0:384])
                    for ko in range(KC):
                        nc.tensor.matmul(
                            pb[:], y[:, ko, bass.ts(m, P)],
                            wout[:, ko, 384:512],
                            start=(ko == 0), stop=(ko == KC - 1))
                    nc.vector.tensor_copy(ot[:, 384:512], pb[:])
                    nc.scalar.dma_start(out[rows, 384:512], ot[:, 384:512])
                else:
                    po = pp_mm.tile([P, C], F32, tag="mm")
                    for ko in range(KC):
                        nc.tensor.matmul(
                            po[:], y[:, ko, bass.ts(m, P)],
                            wout[:, ko, :],
                            start=(ko == 0), stop=(ko == KC - 1))
                    ot = out_pool.tile([P, C], PD)
                    nc.vector.tensor_copy(ot[:], po[:])
                    eng = nc.sync if m % 2 == 0 else nc.scalar
                    eng.dma_start(out[rows, :], ot[:])


def _build_program():
    nc = bass.Bass(trn_type="TRN2", target_bir_lowering=False, debug=False,
                   num_devices=N_CORES)
    xh = nc.dram_tensor("xh", [HEADS, P, KC, LH], PD, kind="ExternalInput").ap()
    wqkv_t = nc.dram_tensor("wqkv_t", [C, 3 * C], PD, kind="ExternalInput").ap()
    wout_t = nc.dram_tensor("wout_t", [C, C], PD, kind="ExternalInput").ap()
    out = nc.dram_tensor("out", [L, C], PD, kind="ExternalOutput").ap()

    from contextlib import ExitStack
    with tile.TileContext(nc) as tc:
        with ExitStack() as ctx:
            _emit(ctx, tc, xh, wqkv_t, wout_t, out)
    _split_wide_waits(nc)
    return nc


def _host_inputs(x, w_qkv, w_out):
    """Per-core input maps. Permute l so head h owns rows [h*512, (h+1)*512)
    (original row i*8+h -> permuted row h*512+i), then lay x out per-head
    transposed, partition-major: xh[h, p, ko, l] = x_perm[h*512+l, ko*128+p],
    so each per-(h,ko) DMA piece is 128 descriptors of contiguous 2 KiB and
    a whole-head load is 128 descriptors of 8 KiB."""
    import ml_dtypes
    bf16 = ml_dtypes.bfloat16
    wqkv_t = np.ascontiguousarray(w_qkv.T).astype(np.float32).copy()
    wqkv_t[:, 0:C] *= SCALE  # fold the attention scale into the Q weights
    wqkv_t = wqkv_t.astype(bf16)
    wout_t = np.ascontiguousarray(w_out.T).astype(bf16)
    in_maps = []
    for b in range(B):
        xb = x[b]  # (L, C); row l = i*8 + h
        x_perm = xb.reshape(LH, HEADS, C).transpose(1, 0, 2)  # (h, lh, c)
        xh = np.ascontiguousarray(
            x_perm.transpose(0, 2, 1)          # (h, c, lh)
            .reshape(HEADS, KC, P, LH)         # c = ko*128 + p
            .transpose(0, 2, 1, 3)).astype(bf16)  # (h, p, ko, lh)
        in_maps.append({"xh": xh, "wqkv_t": wqkv_t, "wout_t": wout_t})
    return in_maps


def _unpermute(out_perm):
    """(L, C) with rows grouped by head -> original row order i*8+h."""
    return out_perm.reshape(HEADS, LH, C).transpose(1, 0, 2).reshape(L, C)


def kernel(x, w_qkv, w_out, b_out, _run_kwargs=None):
    x = np.asarray(x, dtype=np.float32)
    w_qkv = np.asarray(w_qkv, dtype=np.float32)
    w_out = np.asarray(w_out, dtype=np.float32)
    b_out = np.asarray(b_out, dtype=np.float32)

    nc = _build_program()
    in_maps = _host_inputs(x, w_qkv, w_out)
    res = run_bass_kernel_spmd(nc, in_maps, list(range(N_CORES)),
                               **(_run_kwargs or {}))
    out = np.empty((B, L, C), dtype=np.float32)
    for b in range(B):
        out[b] = _unpermute(res.results[b]["out"].astype(np.float32))
    out += b_out
    if _run_kwargs:
        kernel.last_result = res
    return out
